# revision 35
# baseline (speedup 1.0000x reference)
"""Deformable 3D conv (offset-predicting conv + trilinear-sampled 3x3x3 deform conv)
on 8 TRN2 NeuronCores.

Strategy: shard the output D axis (4 planes/core). Per core:
  1. Offset conv as 9 kw-packed fp16 matmuls/vtile (K=96 via 3 w-shifted
     input copies on partitions).
  2. p-pipeline on DVE: clip, floor, fracs; gather indices folded into the
     dma_gather int16 wrap layout via 8 one-hot PE matmuls.
  3. Corner-block table in DRAM: one 512B row per padded voxel holding the
     4 (h,w)-corners of its d-plane plus the 4 d-DIFF corners (fp16), built
     via xbar DMA transposes (the diff kills the lerp d-subtract).
  4. Batched gather: 4 dma_gather calls per 128-voxel chunk, rotated over
     the 4 SWDGE queues so 4 Q7 core pairs emit descriptors in parallel.
  5. Trilinear lerp on DVE, fp16, with fracs pre-broadcast by the scalar
     engine so the multiplies hit DVE 2x mode; d-stage uses table diffs.
  6. Per-chunk PE transposes (identity matmul) + 7 accumulated PE matmuls
     contract (n, c) -> out[64, v]; scalar engine moves PSUM->SBUF.
"""
import os
import sys

for _p in ('/opt/trn_rl_repo', '/root/.axon_site/_ro/trn_rl_repo'):
    if os.path.isdir(_p) and _p not in sys.path:
        sys.path.insert(0, _p)

import numpy as np
import ml_dtypes  # noqa

import concourse.bass as bass
import concourse.mybir as mybir
import concourse.tile as tile
from concourse import bacc
from concourse.bass import IndirectOffsetOnAxis
from concourse.bass_utils import run_bass_kernel_spmd
from concourse.masks import make_identity

F32 = mybir.dt.float32
F16 = mybir.dt.float16
I32 = mybir.dt.int32
I16 = mybir.dt.int16
AL = mybir.AluOpType

# ---------------- problem constants ----------------
C = 32          # input channels
O = 64          # output channels
NN = 27         # kernel sample points
NCORES = 8
DSH = 4         # d-planes per core
V = DSH * 32 * 32   # voxels per core = 4096
P35 = 35
PL = 16         # XE d-planes per core
PLSZ = P35 * P35    # 1225
XE_ROWS = PL * PLSZ  # 19600
TROWS = 19712        # 154 * 128 (padded table rows)
GRPS = TROWS // 128  # 154
XE_FREE = 22400      # >= TROWS + max shift (1191) = 20903; keep 128-mult slack
NVT = 8              # conv v-tiles of 512
NVC = 32             # combine v-chunks of 128

_PROGRAM_CACHE = {}
DEBUG_PIPE = False


def _build_program():
    nc = bacc.Bacc("TRN2", target_bir_lowering=False, debug=False,
                   num_swdge_queues=4)

    xe_d = nc.dram_tensor("xe", [3 * C, XE_FREE], F16, kind="ExternalInput").ap()
    pc_d = nc.dram_tensor("pc", [128, NVC * 96], F32, kind="ExternalInput").ap()
    dclip_d = nc.dram_tensor("dclip", [128, 2], F32, kind="ExternalInput").ap()
    wp_d = nc.dram_tensor("wp", [96, 9 * 96], F16, kind="ExternalInput").ap()
    sel_d = nc.dram_tensor("sel", [128, 8 * 128], F32, kind="ExternalInput").ap()
    wd_d = nc.dram_tensor("wd", [128, 7 * O], F16, kind="ExternalInput").ap()
    out_d = nc.dram_tensor("out_sl", [O, V], F32, kind="ExternalOutput").ap()
    if DEBUG_PIPE:
        dbg_p = nc.dram_tensor("dbg_p", [128, NVC * 96], F32, kind="ExternalOutput").ap()
        dbg_q = nc.dram_tensor("dbg_q", [128, NVC * 96], F32, kind="ExternalOutput").ap()
        dbg_i = nc.dram_tensor("dbg_i", [128, NVC * 27], I32, kind="ExternalOutput").ap()
        dbg_f = nc.dram_tensor("dbg_f", [128, NVC * 96], F16, kind="ExternalOutput").ap()
    t_d = nc.dram_tensor("tdram", [TROWS, 256], F16).ap()

    with tile.TileContext(nc) as tc:
        with tc.tile_pool(name="const", bufs=1) as cpool:
            ident = cpool.tile([128, 128], F32)
            make_identity(nc, ident[:])
            ident16 = cpool.tile([128, 128], F16)
            make_identity(nc, ident16[:])
            wp_sb = cpool.tile([96, 9 * 96], F16)
            nc.sync.dma_start(wp_sb[:], wp_d)
            wd_sb = cpool.tile([128, 7 * O], F16)
            nc.sync.dma_start(wd_sb[:], wd_d)
            pc_sb = cpool.tile([128, NVC * 96], F32)
            nc.sync.dma_start(pc_sb[:], pc_d)
            dclip_sb = cpool.tile([128, 2], F32)
            nc.sync.dma_start(dclip_sb[:], dclip_d)
            # fp16 frac + folded int16 gather indices kept for whole kernel
            frac_t = cpool.tile([128, NVC * 96], F16)
            idxs_all = cpool.tile([128, NVC * 216], I16)

            with tc.tile_pool(name="xe", bufs=1) as xepool:
                # rows [b*32+c] = xe[c] shifted by b in w (for conv K-packing);
                # rows 0-31 are the unshifted copy used everywhere else
                xe4 = xepool.tile([3 * C, XE_FREE], F16)
                nc.sync.dma_start(xe4[:, :], xe_d)

                # ---------- phase 2: corner-block table ----------
                # rows hold [d-plane corners (e=0..3), d-DIFF corners (e=4..7)]
                # so the lerp d-stage needs no subtract
                with (
                    tc.tile_pool(name="xedp", bufs=1) as xedpool,
                    tc.tile_pool(name="tbl", bufs=2) as tpool,
                ):
                    xed = xedpool.tile([32, XE_FREE], F16, tag="xed")
                    nc.vector.tensor_sub(
                        xed[:, 0:XE_FREE - PLSZ],
                        xe4[0:32, PLSZ:XE_FREE],
                        xe4[0:32, 0:XE_FREE - PLSZ])
                    for half in range(2):
                        t_sb = tpool.tile([128, 77 * 256], F16, tag="tsb")
                        base = half * 77 * 128
                        for ed in range(2):
                            for eh in range(2):
                                for ew in range(2):
                                    e = ed * 4 + eh * 2 + ew
                                    dlt = eh * P35 + ew
                                    srct = xed if ed else xe4
                                    src = srct[0:32, base + dlt: base + dlt + 77 * 128]
                                    dst = t_sb[:].rearrange(
                                        "p (g x) -> p g x", x=256
                                    )[:, :, e * 32:(e + 1) * 32]
                                    nc.sync.dma_start_transpose(out=dst, in_=src)
                        # DRAM layout: row' = p*GRPS + g  (contiguous per partition)
                        dst_ap = t_d.rearrange("(p G) x -> p G x", G=GRPS)[
                            :, half * 77:(half + 1) * 77, :]
                        nc.sync.dma_start(
                            out=dst_ap,
                            in_=t_sb[:].rearrange("p (g x) -> p g x", x=256))

                # ---------- phase 3: offset conv ----------
                with (
                    tc.tile_pool(name="convps", bufs=2, space="PSUM") as cps,
                    tc.tile_pool(name="trps", bufs=2, space="PSUM") as tps,
                    tc.tile_pool(name="pipe", bufs=1) as pipe,
                ):
                    sel_sb = pipe.tile([128, 8 * 128], F32)
                    nc.sync.dma_start(sel_sb[:], sel_d)
                    off_sb = pipe.tile([96, NVT * 512], F32)
                    for vt in range(NVT):
                        dl, hh = vt // 2, vt % 2
                        psc = cps.tile([96, 512], F32, tag="convps")
                        for g in range(9):
                            kd, kh = g // 3, g % 3
                            b0 = (dl + kd + 5) * PLSZ + (hh * 16 + kh) * P35
                            rhs = xe4[:, b0:b0 + 16 * P35].rearrange(
                                "p (a b) -> p a b", b=P35)[:, :, 0:32]
                            nc.tensor.matmul(
                                psc[:, :],
                                lhsT=wp_sb[0:96, g * 96:(g + 1) * 96],
                                rhs=rhs,
                                start=(g == 0),
                                stop=(g == 8),
                            )
                        nc.scalar.copy(off_sb[:, vt * 512:(vt + 1) * 512], psc[:, :])

                    # transpose [96, 128]-chunks -> [128, 96] and add pc
                    p_t = pipe.tile([128, NVC * 96], F32)
                    for ch in range(NVC):
                        ptp = tps.tile([128, 96], F32, tag="trps")
                        nc.tensor.transpose(
                            ptp[:, :],
                            off_sb[:, ch * 128:(ch + 1) * 128],
                            ident[0:96, 0:96],
                        )
                        nc.vector.tensor_add(
                            p_t[:, ch * 96:(ch + 1) * 96], ptp[:, :],
                            pc_sb[:, ch * 96:(ch + 1) * 96])

                    # ---------- p-pipeline ----------
                    dv = p_t[:].rearrange("p (v x) -> p v x", x=96)[:, :, 0:27]
                    hwv = p_t[:].rearrange("p (v x) -> p v x", x=96)[:, :, 32:91]
                    # d-axis clip to [lo, hi] (per-core values via dclip)
                    nc.vector.scalar_tensor_tensor(
                        out=dv, in0=dv, scalar=dclip_sb[:, 0:1],
                        in1=dclip_sb[:, 1:2].rearrange(
                            "p (a b) -> p a b", b=1).to_broadcast((128, NVC, 27)),
                        op0=AL.max, op1=AL.min)
                    # h/w clip to [0, 33] (includes junk cols, harmless)
                    nc.vector.tensor_scalar(
                        out=hwv, in0=hwv, scalar1=0.0, scalar2=33.0,
                        op0=AL.max, op1=AL.min)

                    q0i = pipe.tile([128, NVC * 96], I32)
                    nc.vector.tensor_copy(q0i[:], p_t[:])
                    q0f = pipe.tile([128, NVC * 96], F32)
                    nc.vector.tensor_copy(q0f[:], q0i[:])
                    # guard against round-to-nearest casts: q0f -= (q0f > p)
                    fixt = pipe.tile([128, NVC * 96], F32)
                    nc.vector.tensor_tensor(out=fixt[:], in0=q0f[:], in1=p_t[:],
                                            op=AL.is_gt)
                    nc.vector.tensor_sub(q0f[:], q0f[:], fixt[:])
                    # frac (fp16)
                    nc.vector.tensor_sub(frac_t[:], p_t[:], q0f[:])
                    # d-axis safety clamp to [0, 14]
                    q0dv = q0f[:].rearrange("p (v x) -> p v x", x=96)[:, :, 0:27]
                    nc.vector.tensor_scalar(
                        out=q0dv, in0=q0dv, scalar1=0.0, scalar2=14.0,
                        op0=AL.max, op1=AL.min)

                    # idx = (q0d*1225 + q0h)*35 + q0w  (row-major local row r)
                    q0hv = q0f[:].rearrange("p (v x) -> p v x", x=96)[:, :, 32:59]
                    q0wv = q0f[:].rearrange("p (v x) -> p v x", x=96)[:, :, 64:91]
                    idxf = pipe.tile([128, NVC * 27], F32)
                    iv = idxf[:].rearrange("p (v x) -> p v x", x=27)
                    nc.vector.scalar_tensor_tensor(
                        out=iv, in0=q0dv, scalar=35.0, in1=q0hv,
                        op0=AL.mult, op1=AL.add)
                    nc.vector.scalar_tensor_tensor(
                        out=iv, in0=iv, scalar=35.0, in1=q0wv,
                        op0=AL.mult, op1=AL.add)
                    # remap r -> r' = (r % 128)*GRPS + (r // 128)
                    rg = pipe.tile([128, NVC * 27], F32)
                    nc.vector.tensor_scalar_mul(rg[:], idxf[:], 1.0 / 128.0)
                    rgi = pipe.tile([128, NVC * 27], I32)
                    nc.vector.tensor_copy(rgi[:], rg[:])
                    rgf = pipe.tile([128, NVC * 27], F32)
                    nc.vector.tensor_copy(rgf[:], rgi[:])
                    fix2 = pipe.tile([128, NVC * 27], F32)
                    nc.vector.tensor_tensor(out=fix2[:], in0=rgf[:], in1=rg[:],
                                            op=AL.is_gt)
                    nc.vector.tensor_sub(rgf[:], rgf[:], fix2[:])
                    # rp = r - 128*g ; r' = rp*GRPS + g
                    nc.vector.scalar_tensor_tensor(
                        out=idxf[:], in0=rgf[:], scalar=-128.0, in1=idxf[:],
                        op0=AL.mult, op1=AL.add)
                    nc.vector.scalar_tensor_tensor(
                        out=idxf[:], in0=idxf[:], scalar=float(GRPS), in1=rgf[:],
                        op0=AL.mult, op1=AL.add)
                    # fold idxf [128=(ph,p16), (vc,n)] into the dma_gather
                    # int16 wrap layout: idxs_all[p16(all 8 reps), (vc,n)*8+ph]
                    # = idxf[ph*16+p16, (vc,n)], via 8 one-hot matmuls.
                    iv8 = idxs_all[:].rearrange("p (m e) -> p m e", e=8)
                    NF = NVC * 27 // 2  # 432 cols: psum tile fits one bank
                    for ph in range(8):
                        for hf in range(2):
                            psf = tps.tile([128, NF], F32, tag="foldps")
                            nc.tensor.matmul(
                                psf[:, :],
                                lhsT=sel_sb[:, ph * 128:(ph + 1) * 128],
                                rhs=idxf[:, hf * NF:(hf + 1) * NF],
                                start=True, stop=True)
                            nc.vector.tensor_copy(
                                iv8[:, hf * NF:(hf + 1) * NF, ph], psf[:, :])
                    if DEBUG_PIPE:
                        nc.sync.dma_start(dbg_p, p_t[:])
                        nc.sync.dma_start(dbg_q, q0f[:])
                        nc.sync.dma_start(dbg_f, frac_t[:])

            # ---------- phase 4: gather + lerp + contract ----------
            if DEBUG_PIPE:
                pass
            else:
              with (
                  tc.tile_pool(name="gat", bufs=8) as gpool,
                  tc.tile_pool(name="lerp", bufs=3) as lpool,
                  tc.tile_pool(name="accp", bufs=3) as apool,
                  tc.tile_pool(name="ops", bufs=4, space="PSUM") as ops,
                  tc.tile_pool(name="outp", bufs=4) as opool,
              ):
                  # pass 1: gather + lerp per chunk, stream acc to DRAM
                  TAPSPLIT = [(0, 7), (7, 14), (14, 21), (21, 27)]
                  for vc in range(NVC):
                      rt = gpool.tile([128, NN * 256], F16, tag="rt")
                      rt3 = rt[:].rearrange("p (n x) -> p n x", x=256)
                      # 4 queue-parallel dma_gathers fill the 27 tap blocks:
                      # sample i = n*128 + p -> rt[p, n-block], idx wrap [16, s]
                      for q, (t0, t1) in enumerate(TAPSPLIT):
                          nidx = (t1 - t0) * 128
                          nc.gpsimd.dma_gather(
                              rt3[:, t0:t1, :],
                              t_d,
                              idxs_all[:, vc * 216 + t0 * 8:vc * 216 + t1 * 8],
                              nidx,
                              nidx,
                              256,
                              queue_num=q,
                          )
                      rv = rt[:].rearrange("p (n x) -> p n x", x=256)

                      def _fb(col, rep):
                          s = frac_t[:, vc * 96 + col: vc * 96 + col + 27]
                          return s.rearrange("p (n o) -> p n o", o=1).to_broadcast(
                              (128, NN, rep))

                      # expand fracs to contiguous fp16 on ACT so the lerp
                      # multiplies run in DVE 2x mode instead of 1x broadcast
                      fxp = lpool.tile([128, NN * 224], F16, tag="fxp")
                      fdx = fxp[:, :NN * 128].rearrange("p (n x) -> p n x", x=128)
                      fhx = fxp[:, NN * 128:NN * 192].rearrange(
                          "p (n x) -> p n x", x=64)
                      fwx = fxp[:, NN * 192:].rearrange("p (n x) -> p n x", x=32)
                      nc.scalar.copy(fdx, _fb(0, 128))
                      nc.scalar.copy(fhx, _fb(32, 64))
                      nc.scalar.copy(fwx, _fb(64, 32))

                      # d-lerp: high half of rt already holds the d-diffs
                      av = rv[:, :, 128:256]
                      nc.vector.tensor_tensor(out=av, in0=av, in1=fdx, op=AL.mult)
                      nc.vector.tensor_add(av, av, rv[:, :, 0:128])

                      bv = av[:, :, 64:128]
                      nc.vector.tensor_sub(bv, av[:, :, 64:128], av[:, :, 0:64])
                      nc.vector.tensor_tensor(out=bv, in0=bv, in1=fhx, op=AL.mult)
                      nc.vector.tensor_add(bv, bv, av[:, :, 0:64])

                      ct = apool.tile([128, 896], F16, tag="ct")
                      nc.vector.memset(ct[:, 864:896], 0.0)
                      cv = ct[:, 0:NN * 32].rearrange("p (n x) -> p n x", x=32)
                      nc.vector.tensor_sub(cv, bv[:, :, 32:64], bv[:, :, 0:32])
                      nc.vector.tensor_tensor(out=cv, in0=cv, in1=fwx, op=AL.mult)
                      nc.vector.tensor_add(cv, cv, bv[:, :, 0:32])

                      # transpose acc via PE (keeps xbar/DMA free for gathers),
                      # then contract and write out
                      acct = apool.tile([128, 7, 128], F16, tag="acct")
                      for g in range(7):
                          trp = ops.tile([128, 128], F16, tag="trp")
                          nc.tensor.transpose(
                              trp[:, :], ct[:, g * 128:(g + 1) * 128],
                              ident16[0:128, 0:128])
                          nc.scalar.copy(acct[:, g, :], trp[:, :])
                      pso = ops.tile([64, 128], F32, tag="pso")
                      for g in range(7):
                          nc.tensor.matmul(
                              pso[:, :],
                              lhsT=wd_sb[:, g * O:(g + 1) * O],
                              rhs=acct[:, g, :],
                              start=(g == 0), stop=(g == 6))
                      osb = opool.tile([64, 128], F32, tag="osb")
                      nc.scalar.copy(osb[:], pso[:, :])
                      nc.sync.dma_start(
                          out=out_d[:, vc * 128:(vc + 1) * 128], in_=osb[:])

    nc.compile()
    return nc


def _host_prep(x, w_p, b_p, w_d):
    """Build per-core input maps."""
    x = np.asarray(x, np.float32)
    w_p = np.asarray(w_p, np.float32)
    b_p = np.asarray(b_p, np.float32)
    w_d = np.asarray(w_d, np.float32)

    # global padded/extended volume, channel-first, fp16:
    # XG[c, g, h', w'] with g = xp_plane + 5 (xp planes -5..39), h', w' in [0,35)
    XG = np.zeros((C, 45, P35, P35), np.float16)
    XG[:, 6:38, 1:33, 1:33] = x[0].astype(np.float16)

    # pc (shared): [128, 32*96] f32
    v = np.arange(V)
    dl, hh, wl = v >> 10, (v >> 5) & 31, v & 31
    r = np.array([-1.0, 0.0, 1.0], np.float32)
    pn_d, pn_h, pn_w = np.meshgrid(r, r, r, indexing='ij')
    pn = np.stack([pn_d.ravel(), pn_h.ravel(), pn_w.ravel()])  # (3, 27)
    pc = np.zeros((V, 96), np.float32)
    pc[:, 0:27] = (dl[:, None] + 6.0) + pn[0][None, :] + b_p[None, 0:27]
    pc[:, 32:59] = (hh[:, None] + 1.0) + pn[1][None, :] + b_p[None, 27:54]
    pc[:, 64:91] = (wl[:, None] + 1.0) + pn[2][None, :] + b_p[None, 54:81]
    pc_t = pc.reshape(NVC, 128, 96).transpose(1, 0, 2).reshape(128, NVC * 96)
    pc_t = np.ascontiguousarray(pc_t, np.float32)

    # wp lhsT: [96, 9*96] fp16; K packs (kw-shift b, c), one 96-col slice
    # per (kd, kh) group
    wp_l = np.zeros((96, 9 * 96), np.float16)
    colmap = np.full(96, -1, np.int64)
    colmap[0:27] = np.arange(27)
    colmap[32:59] = 27 + np.arange(27)
    colmap[64:91] = 54 + np.arange(27)
    for g in range(9):
        kd, kh = g // 3, g % 3
        for b in range(3):
            for m in range(96):
                ch = colmap[m]
                if ch < 0:
                    continue
                wp_l[b * 32:(b + 1) * 32, g * 96 + m] = w_p[ch, :, kd, kh, b]

    # wd lhsT: [128, 7*64] fp16 (K-row (g, pk): n = 4g + pk//32, c = pk%32)
    wd_l = np.zeros((128, 7 * O), np.float16)
    for g in range(7):
        for pk in range(128):
            n = 4 * g + pk // 32
            if n >= NN:
                continue
            wd_l[pk, g * O:(g + 1) * O] = w_d[:, pk % 32, n // 9, (n // 3) % 3, n % 3]

    # sel: 8 one-hot fold matrices [k=128, m=128]; sel[ph][k, m] = 1 iff
    # k == ph*16 + (m % 16)  (lhsT layout, packed side by side)
    sel = np.zeros((128, 8 * 128), np.float32)
    for ph in range(8):
        for m in range(128):
            sel[ph * 16 + (m % 16), ph * 128 + m] = 1.0

    in_maps = []
    for k in range(NCORES):
        dlo = 4 * k - 5
        xe = np.zeros((3 * C, XE_FREE), np.float16)
        flat = XG[:, 4 * k:4 * k + PL].reshape(C, XE_ROWS)
        xe[0:C, :XE_ROWS] = flat
        xe[C:2 * C, :XE_ROWS - 1] = flat[:, 1:]
        xe[2 * C:3 * C, :XE_ROWS - 2] = flat[:, 2:]
        dclip = np.zeros((128, 2), np.float32)
        dclip[:, 0] = 0.0 - dlo
        dclip[:, 1] = 33.0 - dlo
        in_maps.append({
            "xe": xe,
            "pc": pc_t,
            "dclip": dclip,
            "wp": wp_l,
            "wd": wd_l,
            "sel": sel,
        })
    return in_maps


def kernel(x, w_p, b_p, w_d):
    if "nc" not in _PROGRAM_CACHE:
        _PROGRAM_CACHE["nc"] = _build_program()
    nc = _PROGRAM_CACHE["nc"]
    in_maps = _host_prep(x, w_p, b_p, w_d)
    res = run_bass_kernel_spmd(nc, in_maps, list(range(NCORES))).results
    out = np.empty((1, O, 32, 32, 32), np.float32)
    for k in range(NCORES):
        out[0, :, 4 * k:4 * k + 4] = res[k]["out_sl"].reshape(O, DSH, 32, 32)
    return out



# revision 38
# speedup vs baseline: 1.0383x; 1.0383x over previous
"""Deformable 3D conv (offset-predicting conv + trilinear-sampled 3x3x3 deform conv)
on 8 TRN2 NeuronCores.

Strategy: shard the output D axis (4 planes/core). Per core:
  1. Offset conv as 9 kw-packed fp16 matmuls/vtile (K=96 via 3 w-shifted
     input copies on partitions).
  2. p-pipeline on DVE: clip, floor, fracs; gather indices folded into the
     dma_gather int16 wrap layout via 8 one-hot PE matmuls.
  3. Corner-block table in DRAM: one 512B row per padded voxel holding the
     4 (h,w)-corners of its d-plane plus the 4 d-DIFF corners (fp16), built
     via xbar DMA transposes (the diff kills the lerp d-subtract).
  4. Batched gather: 4 dma_gather calls per 128-voxel chunk, rotated over
     the 4 SWDGE queues so 4 Q7 core pairs emit descriptors in parallel.
  5. Trilinear lerp on DVE, fp16, with fracs pre-broadcast by the scalar
     engine so the multiplies hit DVE 2x mode; d-stage uses table diffs.
  6. Per-chunk PE transposes (identity matmul) + 7 accumulated PE matmuls
     contract (n, c) -> out[64, v]; scalar engine moves PSUM->SBUF.
"""
import os
import sys

for _p in ('/opt/trn_rl_repo', '/root/.axon_site/_ro/trn_rl_repo'):
    if os.path.isdir(_p) and _p not in sys.path:
        sys.path.insert(0, _p)

import numpy as np
import ml_dtypes  # noqa

import concourse.bass as bass
import concourse.mybir as mybir
import concourse.tile as tile
from concourse import bacc
from concourse.bass import IndirectOffsetOnAxis
from concourse.bass_utils import run_bass_kernel_spmd
from concourse.masks import make_identity

F32 = mybir.dt.float32
F16 = mybir.dt.float16
I32 = mybir.dt.int32
I16 = mybir.dt.int16
AL = mybir.AluOpType

# ---------------- problem constants ----------------
C = 32          # input channels
O = 64          # output channels
NN = 27         # kernel sample points
NCORES = 8
DSH = 4         # d-planes per core
V = DSH * 32 * 32   # voxels per core = 4096
P35 = 35
PL = 16         # XE d-planes per core
PLSZ = P35 * P35    # 1225
XE_ROWS = PL * PLSZ  # 19600
TROWS = 19712        # 154 * 128 (padded table rows)
GRPS = TROWS // 128  # 154
XE_FREE = 22400      # >= TROWS + max shift (1191) = 20903; keep 128-mult slack
NVT = 8              # conv v-tiles of 512
NVC = 32             # combine v-chunks of 128

_PROGRAM_CACHE = {}
DEBUG_PIPE = False


def _build_program():
    nc = bacc.Bacc("TRN2", target_bir_lowering=False, debug=False,
                   num_swdge_queues=4)

    xe_d = nc.dram_tensor("xe", [3 * C, XE_FREE], F16, kind="ExternalInput").ap()
    pc_d = nc.dram_tensor("pc", [128, NVC * 96], F32, kind="ExternalInput").ap()
    dclip_d = nc.dram_tensor("dclip", [128, 2], F32, kind="ExternalInput").ap()
    wp_d = nc.dram_tensor("wp", [96, 9 * 96], F16, kind="ExternalInput").ap()
    sel_d = nc.dram_tensor("sel", [128, 8 * 128], F32, kind="ExternalInput").ap()
    wd_d = nc.dram_tensor("wd", [128, 7 * O], F16, kind="ExternalInput").ap()
    out_d = nc.dram_tensor("out_sl", [O, V], F32, kind="ExternalOutput").ap()
    if DEBUG_PIPE:
        dbg_p = nc.dram_tensor("dbg_p", [128, NVC * 96], F32, kind="ExternalOutput").ap()
        dbg_q = nc.dram_tensor("dbg_q", [128, NVC * 96], F32, kind="ExternalOutput").ap()
        dbg_i = nc.dram_tensor("dbg_i", [128, NVC * 27], I32, kind="ExternalOutput").ap()
        dbg_f = nc.dram_tensor("dbg_f", [128, NVC * 96], F16, kind="ExternalOutput").ap()
    t_d = nc.dram_tensor("tdram", [TROWS, 256], F16).ap()

    with tile.TileContext(nc) as tc:
        with tc.tile_pool(name="const", bufs=1) as cpool:
            ident = cpool.tile([128, 128], F32)
            make_identity(nc, ident[:])
            ident16 = cpool.tile([128, 128], F16)
            make_identity(nc, ident16[:])
            wp_sb = cpool.tile([96, 9 * 96], F16)
            nc.sync.dma_start(wp_sb[:], wp_d)
            wd_sb = cpool.tile([128, 7 * O], F16)
            nc.sync.dma_start(wd_sb[:], wd_d)
            pc_sb = cpool.tile([128, NVC * 96], F32)
            nc.sync.dma_start(pc_sb[:], pc_d)
            dclip_sb = cpool.tile([128, 2], F32)
            nc.sync.dma_start(dclip_sb[:], dclip_d)
            # fp16 frac + folded int16 gather indices kept for whole kernel
            frac_t = cpool.tile([128, NVC * 96], F16)
            idxs_all = cpool.tile([128, NVC * 216], I16)

            with (
                tc.tile_pool(name="xe", bufs=1) as xepool,
                tc.tile_pool(name="convp", bufs=1) as convpool,
                tc.tile_pool(name="convps", bufs=2, space="PSUM") as cps,
            ):
                # rows [b*32+c] = xe[c] shifted by b in w (for conv K-packing);
                # rows 0-31 are the unshifted copy used everywhere else
                xe4 = xepool.tile([3 * C, XE_FREE], F16)
                nc.sync.dma_start(xe4[:, :], xe_d)

                # offset conv first: PE work overlaps the table-build DMAs
                off_sb = convpool.tile([96, NVT * 512], F32)
                for vt in range(NVT):
                    dl, hh = vt // 2, vt % 2
                    psc = cps.tile([96, 512], F32, tag="convps")
                    for g in range(9):
                        kd, kh = g // 3, g % 3
                        b0 = (dl + kd + 5) * PLSZ + (hh * 16 + kh) * P35
                        rhs = xe4[:, b0:b0 + 16 * P35].rearrange(
                            "p (a b) -> p a b", b=P35)[:, :, 0:32]
                        nc.tensor.matmul(
                            psc[:, :],
                            lhsT=wp_sb[0:96, g * 96:(g + 1) * 96],
                            rhs=rhs,
                            start=(g == 0),
                            stop=(g == 8),
                        )
                    nc.scalar.copy(off_sb[:, vt * 512:(vt + 1) * 512], psc[:, :])

                # ---------- phase 2: corner-block table ----------
                # rows hold [d-plane corners (e=0..3), d-DIFF corners (e=4..7)]
                # so the lerp d-stage needs no subtract
                with (
                    tc.tile_pool(name="xedp", bufs=1) as xedpool,
                    tc.tile_pool(name="tbl", bufs=1) as tpool,
                ):
                    xed = xedpool.tile([32, XE_FREE], F16, tag="xed")
                    nc.vector.tensor_sub(
                        xed[:, 0:XE_FREE - PLSZ],
                        xe4[0:32, PLSZ:XE_FREE],
                        xe4[0:32, 0:XE_FREE - PLSZ])
                    for half in range(2):
                        t_sb = tpool.tile([128, 77 * 256], F16, tag="tsb")
                        base = half * 77 * 128
                        for ed in range(2):
                            for eh in range(2):
                                for ew in range(2):
                                    e = ed * 4 + eh * 2 + ew
                                    dlt = eh * P35 + ew
                                    srct = xed if ed else xe4
                                    src = srct[0:32, base + dlt: base + dlt + 77 * 128]
                                    dst = t_sb[:].rearrange(
                                        "p (g x) -> p g x", x=256
                                    )[:, :, e * 32:(e + 1) * 32]
                                    nc.sync.dma_start_transpose(out=dst, in_=src)
                        # DRAM layout: row' = p*GRPS + g  (contiguous per partition)
                        dst_ap = t_d.rearrange("(p G) x -> p G x", G=GRPS)[
                            :, half * 77:(half + 1) * 77, :]
                        nc.scalar.dma_start(
                            out=dst_ap,
                            in_=t_sb[:].rearrange("p (g x) -> p g x", x=256))

                # ---------- phase 3: p-pipeline ----------
                with (
                    tc.tile_pool(name="trps", bufs=2, space="PSUM") as tps,
                    tc.tile_pool(name="pipe", bufs=1) as pipe,
                ):
                    sel_sb = pipe.tile([128, 8 * 128], F32)
                    nc.sync.dma_start(sel_sb[:], sel_d)

                    # transpose [96, 128]-chunks -> [128, 96] and add pc
                    p_t = pipe.tile([128, NVC * 96], F32)
                    for ch in range(NVC):
                        ptp = tps.tile([128, 96], F32, tag="trps")
                        nc.tensor.transpose(
                            ptp[:, :],
                            off_sb[:, ch * 128:(ch + 1) * 128],
                            ident[0:96, 0:96],
                        )
                        nc.vector.tensor_add(
                            p_t[:, ch * 96:(ch + 1) * 96], ptp[:, :],
                            pc_sb[:, ch * 96:(ch + 1) * 96])

                    # ---------- p-pipeline ----------
                    dv = p_t[:].rearrange("p (v x) -> p v x", x=96)[:, :, 0:27]
                    hwv = p_t[:].rearrange("p (v x) -> p v x", x=96)[:, :, 32:91]
                    # d-axis clip to [lo, hi] (per-core values via dclip)
                    nc.vector.scalar_tensor_tensor(
                        out=dv, in0=dv, scalar=dclip_sb[:, 0:1],
                        in1=dclip_sb[:, 1:2].rearrange(
                            "p (a b) -> p a b", b=1).to_broadcast((128, NVC, 27)),
                        op0=AL.max, op1=AL.min)
                    # h/w clip to [0, 33] (includes junk cols, harmless)
                    nc.vector.tensor_scalar(
                        out=hwv, in0=hwv, scalar1=0.0, scalar2=33.0,
                        op0=AL.max, op1=AL.min)

                    q0i = pipe.tile([128, NVC * 96], I32)
                    nc.vector.tensor_copy(q0i[:], p_t[:])
                    q0f = pipe.tile([128, NVC * 96], F32)
                    nc.vector.tensor_copy(q0f[:], q0i[:])
                    # guard against round-to-nearest casts: q0f -= (q0f > p)
                    fixt = pipe.tile([128, NVC * 96], F32)
                    nc.vector.tensor_tensor(out=fixt[:], in0=q0f[:], in1=p_t[:],
                                            op=AL.is_gt)
                    nc.vector.tensor_sub(q0f[:], q0f[:], fixt[:])
                    # frac (fp16)
                    nc.vector.tensor_sub(frac_t[:], p_t[:], q0f[:])
                    # d-axis safety clamp to [0, 14]
                    q0dv = q0f[:].rearrange("p (v x) -> p v x", x=96)[:, :, 0:27]
                    nc.vector.tensor_scalar(
                        out=q0dv, in0=q0dv, scalar1=0.0, scalar2=14.0,
                        op0=AL.max, op1=AL.min)

                    # idx = (q0d*1225 + q0h)*35 + q0w  (row-major local row r)
                    q0hv = q0f[:].rearrange("p (v x) -> p v x", x=96)[:, :, 32:59]
                    q0wv = q0f[:].rearrange("p (v x) -> p v x", x=96)[:, :, 64:91]
                    idxf = pipe.tile([128, NVC * 27], F32)
                    iv = idxf[:].rearrange("p (v x) -> p v x", x=27)
                    nc.vector.scalar_tensor_tensor(
                        out=iv, in0=q0dv, scalar=35.0, in1=q0hv,
                        op0=AL.mult, op1=AL.add)
                    nc.vector.scalar_tensor_tensor(
                        out=iv, in0=iv, scalar=35.0, in1=q0wv,
                        op0=AL.mult, op1=AL.add)
                    # remap r -> r' = (r % 128)*GRPS + (r // 128)
                    rg = pipe.tile([128, NVC * 27], F32)
                    nc.vector.tensor_scalar_mul(rg[:], idxf[:], 1.0 / 128.0)
                    rgi = pipe.tile([128, NVC * 27], I32)
                    nc.vector.tensor_copy(rgi[:], rg[:])
                    rgf = pipe.tile([128, NVC * 27], F32)
                    nc.vector.tensor_copy(rgf[:], rgi[:])
                    fix2 = pipe.tile([128, NVC * 27], F32)
                    nc.vector.tensor_tensor(out=fix2[:], in0=rgf[:], in1=rg[:],
                                            op=AL.is_gt)
                    nc.vector.tensor_sub(rgf[:], rgf[:], fix2[:])
                    # rp = r - 128*g ; r' = rp*GRPS + g
                    nc.vector.scalar_tensor_tensor(
                        out=idxf[:], in0=rgf[:], scalar=-128.0, in1=idxf[:],
                        op0=AL.mult, op1=AL.add)
                    nc.vector.scalar_tensor_tensor(
                        out=idxf[:], in0=idxf[:], scalar=float(GRPS), in1=rgf[:],
                        op0=AL.mult, op1=AL.add)
                    # fold idxf [128=(ph,p16), (vc,n)] into the dma_gather
                    # int16 wrap layout: idxs_all[p16(all 8 reps), (vc,n)*8+ph]
                    # = idxf[ph*16+p16, (vc,n)], via 8 one-hot matmuls.
                    iv8 = idxs_all[:].rearrange("p (m e) -> p m e", e=8)
                    NF = NVC * 27 // 2  # 432 cols: psum tile fits one bank
                    for ph in range(8):
                        for hf in range(2):
                            psf = tps.tile([128, NF], F32, tag="foldps")
                            nc.tensor.matmul(
                                psf[:, :],
                                lhsT=sel_sb[:, ph * 128:(ph + 1) * 128],
                                rhs=idxf[:, hf * NF:(hf + 1) * NF],
                                start=True, stop=True)
                            nc.vector.tensor_copy(
                                iv8[:, hf * NF:(hf + 1) * NF, ph], psf[:, :])
                    if DEBUG_PIPE:
                        nc.sync.dma_start(dbg_p, p_t[:])
                        nc.sync.dma_start(dbg_q, q0f[:])
                        nc.sync.dma_start(dbg_f, frac_t[:])

            # ---------- phase 4: gather + lerp + contract ----------
            if DEBUG_PIPE:
                pass
            else:
              with (
                  tc.tile_pool(name="gat", bufs=8) as gpool,
                  tc.tile_pool(name="lerp", bufs=3) as lpool,
                  tc.tile_pool(name="accp", bufs=3) as apool,
                  tc.tile_pool(name="ops", bufs=4, space="PSUM") as ops,
                  tc.tile_pool(name="outp", bufs=4) as opool,
              ):
                  # pass 1: gather + lerp per chunk, stream acc to DRAM
                  TAPSPLIT = [(0, 7), (7, 14), (14, 21), (21, 27)]
                  for vc in range(NVC):
                      rt = gpool.tile([128, NN * 256], F16, tag="rt")
                      rt3 = rt[:].rearrange("p (n x) -> p n x", x=256)
                      # 4 queue-parallel dma_gathers fill the 27 tap blocks:
                      # sample i = n*128 + p -> rt[p, n-block], idx wrap [16, s]
                      for q, (t0, t1) in enumerate(TAPSPLIT):
                          nidx = (t1 - t0) * 128
                          nc.gpsimd.dma_gather(
                              rt3[:, t0:t1, :],
                              t_d,
                              idxs_all[:, vc * 216 + t0 * 8:vc * 216 + t1 * 8],
                              nidx,
                              nidx,
                              256,
                              queue_num=q,
                          )
                      rv = rt[:].rearrange("p (n x) -> p n x", x=256)

                      def _fb(col, rep):
                          s = frac_t[:, vc * 96 + col: vc * 96 + col + 27]
                          return s.rearrange("p (n o) -> p n o", o=1).to_broadcast(
                              (128, NN, rep))

                      # expand fracs to contiguous fp16 on ACT so the lerp
                      # multiplies run in DVE 2x mode instead of 1x broadcast
                      fxp = lpool.tile([128, NN * 224], F16, tag="fxp")
                      fdx = fxp[:, :NN * 128].rearrange("p (n x) -> p n x", x=128)
                      fhx = fxp[:, NN * 128:NN * 192].rearrange(
                          "p (n x) -> p n x", x=64)
                      fwx = fxp[:, NN * 192:].rearrange("p (n x) -> p n x", x=32)
                      nc.scalar.copy(fdx, _fb(0, 128))
                      nc.scalar.copy(fhx, _fb(32, 64))
                      nc.scalar.copy(fwx, _fb(64, 32))

                      # d-lerp: high half of rt already holds the d-diffs
                      av = rv[:, :, 128:256]
                      nc.vector.tensor_tensor(out=av, in0=av, in1=fdx, op=AL.mult)
                      nc.vector.tensor_add(av, av, rv[:, :, 0:128])

                      bv = av[:, :, 64:128]
                      nc.vector.tensor_sub(bv, av[:, :, 64:128], av[:, :, 0:64])
                      nc.vector.tensor_tensor(out=bv, in0=bv, in1=fhx, op=AL.mult)
                      nc.vector.tensor_add(bv, bv, av[:, :, 0:64])

                      ct = apool.tile([128, 896], F16, tag="ct")
                      nc.vector.memset(ct[:, 864:896], 0.0)
                      cv = ct[:, 0:NN * 32].rearrange("p (n x) -> p n x", x=32)
                      nc.vector.tensor_sub(cv, bv[:, :, 32:64], bv[:, :, 0:32])
                      nc.vector.tensor_tensor(out=cv, in0=cv, in1=fwx, op=AL.mult)
                      nc.vector.tensor_add(cv, cv, bv[:, :, 0:32])

                      # transpose acc via PE (keeps xbar/DMA free for gathers),
                      # then contract and write out
                      acct = apool.tile([128, 7, 128], F16, tag="acct")
                      for g in range(7):
                          trp = ops.tile([128, 128], F16, tag="trp")
                          nc.tensor.transpose(
                              trp[:, :], ct[:, g * 128:(g + 1) * 128],
                              ident16[0:128, 0:128])
                          nc.scalar.copy(acct[:, g, :], trp[:, :])
                      pso = ops.tile([64, 128], F32, tag="pso")
                      for g in range(7):
                          nc.tensor.matmul(
                              pso[:, :],
                              lhsT=wd_sb[:, g * O:(g + 1) * O],
                              rhs=acct[:, g, :],
                              start=(g == 0), stop=(g == 6))
                      osb = opool.tile([64, 128], F32, tag="osb")
                      nc.scalar.copy(osb[:], pso[:, :])
                      nc.sync.dma_start(
                          out=out_d[:, vc * 128:(vc + 1) * 128], in_=osb[:])

    nc.compile()
    return nc


def _host_prep(x, w_p, b_p, w_d):
    """Build per-core input maps."""
    x = np.asarray(x, np.float32)
    w_p = np.asarray(w_p, np.float32)
    b_p = np.asarray(b_p, np.float32)
    w_d = np.asarray(w_d, np.float32)

    # global padded/extended volume, channel-first, fp16:
    # XG[c, g, h', w'] with g = xp_plane + 5 (xp planes -5..39), h', w' in [0,35)
    XG = np.zeros((C, 45, P35, P35), np.float16)
    XG[:, 6:38, 1:33, 1:33] = x[0].astype(np.float16)

    # pc (shared): [128, 32*96] f32
    v = np.arange(V)
    dl, hh, wl = v >> 10, (v >> 5) & 31, v & 31
    r = np.array([-1.0, 0.0, 1.0], np.float32)
    pn_d, pn_h, pn_w = np.meshgrid(r, r, r, indexing='ij')
    pn = np.stack([pn_d.ravel(), pn_h.ravel(), pn_w.ravel()])  # (3, 27)
    pc = np.zeros((V, 96), np.float32)
    pc[:, 0:27] = (dl[:, None] + 6.0) + pn[0][None, :] + b_p[None, 0:27]
    pc[:, 32:59] = (hh[:, None] + 1.0) + pn[1][None, :] + b_p[None, 27:54]
    pc[:, 64:91] = (wl[:, None] + 1.0) + pn[2][None, :] + b_p[None, 54:81]
    pc_t = pc.reshape(NVC, 128, 96).transpose(1, 0, 2).reshape(128, NVC * 96)
    pc_t = np.ascontiguousarray(pc_t, np.float32)

    # wp lhsT: [96, 9*96] fp16; K packs (kw-shift b, c), one 96-col slice
    # per (kd, kh) group
    wp_l = np.zeros((96, 9 * 96), np.float16)
    colmap = np.full(96, -1, np.int64)
    colmap[0:27] = np.arange(27)
    colmap[32:59] = 27 + np.arange(27)
    colmap[64:91] = 54 + np.arange(27)
    for g in range(9):
        kd, kh = g // 3, g % 3
        for b in range(3):
            for m in range(96):
                ch = colmap[m]
                if ch < 0:
                    continue
                wp_l[b * 32:(b + 1) * 32, g * 96 + m] = w_p[ch, :, kd, kh, b]

    # wd lhsT: [128, 7*64] fp16 (K-row (g, pk): n = 4g + pk//32, c = pk%32)
    wd_l = np.zeros((128, 7 * O), np.float16)
    for g in range(7):
        for pk in range(128):
            n = 4 * g + pk // 32
            if n >= NN:
                continue
            wd_l[pk, g * O:(g + 1) * O] = w_d[:, pk % 32, n // 9, (n // 3) % 3, n % 3]

    # sel: 8 one-hot fold matrices [k=128, m=128]; sel[ph][k, m] = 1 iff
    # k == ph*16 + (m % 16)  (lhsT layout, packed side by side)
    sel = np.zeros((128, 8 * 128), np.float32)
    for ph in range(8):
        for m in range(128):
            sel[ph * 16 + (m % 16), ph * 128 + m] = 1.0

    in_maps = []
    for k in range(NCORES):
        dlo = 4 * k - 5
        xe = np.zeros((3 * C, XE_FREE), np.float16)
        flat = XG[:, 4 * k:4 * k + PL].reshape(C, XE_ROWS)
        xe[0:C, :XE_ROWS] = flat
        xe[C:2 * C, :XE_ROWS - 1] = flat[:, 1:]
        xe[2 * C:3 * C, :XE_ROWS - 2] = flat[:, 2:]
        dclip = np.zeros((128, 2), np.float32)
        dclip[:, 0] = 0.0 - dlo
        dclip[:, 1] = 33.0 - dlo
        in_maps.append({
            "xe": xe,
            "pc": pc_t,
            "dclip": dclip,
            "wp": wp_l,
            "wd": wd_l,
            "sel": sel,
        })
    return in_maps


def kernel(x, w_p, b_p, w_d):
    if "nc" not in _PROGRAM_CACHE:
        _PROGRAM_CACHE["nc"] = _build_program()
    nc = _PROGRAM_CACHE["nc"]
    in_maps = _host_prep(x, w_p, b_p, w_d)
    res = run_bass_kernel_spmd(nc, in_maps, list(range(NCORES))).results
    out = np.empty((1, O, 32, 32, 32), np.float32)
    for k in range(NCORES):
        out[0, :, 4 * k:4 * k + 4] = res[k]["out_sl"].reshape(O, DSH, 32, 32)
    return out

